# revision 35
# baseline (speedup 1.0000x reference)
"""Multi-head attention (B=2, S=2048, D=1024, H=16) on 8 trn2 NeuronCores.

Sharding: data-parallel over batch (2 groups of 4 cores), tensor-parallel over
heads within a group (4 heads/core).  Each core computes a partial output
(its heads' contribution through its W_o column shard); the host sums the 4
partials per batch element.

Schedule (v4): one FLAT 128-step software pipeline over g = (block, k-tile)
with no block boundaries: scores(g) / exp(g-1) / AV(g-2) per step, so the ACT
exp stream (143us total) never waits at a block seam.  The head-pair scores
matmuls at tile_position (0,0)/(64,0) execute CONCURRENTLY on the PE (row-tile
parallelism), so PE totals ~142us - the two engines are co-critical and both
must run dense:
  - background projection work is split into <=0.9us half-units and placed in
    specific pipeline steps, at most ~1 per step outside the DMA-paced head.
  - input DMAs are split across BOTH hardware DGE queues in consumption
    order (sync: wk,K0a,wv,V0,K1..K3,Q1..Q3,wo / scalar: wq,K0b,Q0,V1..V3)
    so the first-block gate is ~1MB per queue and nothing later stalls.
  - warm matmuls pad the PE until K0 lands: ANY PE idle makes the HAM drop
    the clock to 4/8 and it takes ~4us of continuous work to win it back.
  - the denominator-broadcast matmul allocates from the projection PSUM pool
    (not the scores pool) so epilogues never stall the exp stream.
  - tail: the final block's epilogue runs inline with ACT doing the copies,
    and the last 4 output projections drain via ACT+DVE and DMA out over the
    (by then idle) HWDGE queues.

Per-core layout (unchanged from v1):
  - q, k produced TRANSPOSED ([d_local, s]); scores lhsT = kT tile, rhs = qT,
    head pairs row-packed via tile_position (0,0)/(64,0).
  - v in natural layout with a fused ones-column so the AV matmul emits the
    attention output and the softmax denominator in one accumulation chain.
  - softmax skips max-subtraction (scores ~ N(0,1); fp32 exp cannot overflow).
"""

import os
from contextlib import ExitStack

import numpy as np

B = 2
S = 2048
DM = 1024
H = 16
DK = 64
P = 128
HC = 4            # heads per core
DO = HC * DK      # 256: local output dim of q/k/v projections
DI_T = DM // P    # 8 contraction tiles for projections
S_T = S // P      # 16
N_SC = 4          # s-chunks of 512
NB = 8            # blocks = 4 q-chunks x 2 head-pairs
NG = NB * S_T     # 128 pipeline steps

MM_BF16 = True    # matmul inputs in fp16 (fp32 PSUM accumulation everywhere)

_PROGRAM = None


def _build_program():
    import concourse.mybir as mybir
    import concourse.tile as tile
    from concourse import bacc

    f32 = mybir.dt.float32
    mmdt = mybir.dt.float16 if MM_BF16 else f32
    nc = bacc.Bacc("TRN2", target_bir_lowering=False, debug=False)

    # pre-tiled inputs: chunk sc is contiguous ([sc][p][t][512])
    qt_d = nc.dram_tensor("QTC", [N_SC, P, DI_T, 512], mmdt, kind="ExternalInput").ap()
    kt_d = nc.dram_tensor("KTC", [N_SC, P, DI_T, 512], mmdt, kind="ExternalInput").ap()
    vt_d = nc.dram_tensor("VTC", [N_SC, P, DI_T, 512], mmdt, kind="ExternalInput").ap()
    wqt_d = nc.dram_tensor("WQC", [P, DI_T, DO], mmdt, kind="ExternalInput").ap()
    wkt_d = nc.dram_tensor("WKC", [P, DI_T, DO], mmdt, kind="ExternalInput").ap()
    wvt_d = nc.dram_tensor("WVC", [P, DI_T, DO], mmdt, kind="ExternalInput").ap()
    wot_d = nc.dram_tensor("WOC", [P, 2, DM], mmdt, kind="ExternalInput").ap()
    out_d = nc.dram_tensor("OUT", [S, DM], mmdt, kind="ExternalOutput").ap()

    with tile.TileContext(nc) as tc, ExitStack() as ctx:
        _emit(ctx, tc, qt_d, kt_d, vt_d, wqt_d, wkt_d, wvt_d, wot_d, out_d)
    nc.compile()
    return nc


def _emit(ctx, tc, qt_d, kt_d, vt_d, wqt_d, wkt_d, wvt_d, wot_d, out_d):
    import concourse.mybir as mybir

    nc = tc.nc
    f32 = mybir.dt.float32
    mmdt = mybir.dt.float16 if MM_BF16 else f32
    Exp = mybir.ActivationFunctionType.Exp

    consts = ctx.enter_context(tc.tile_pool(name="consts", bufs=1))
    exp_pool = ctx.enter_context(tc.tile_pool(name="exp", bufs=4))
    smalls = ctx.enter_context(tc.tile_pool(name="smalls", bufs=2))
    avdrain = ctx.enter_context(tc.tile_pool(name="avdrain", bufs=2))
    ostage = ctx.enter_context(tc.tile_pool(name="ostage", bufs=4))

    # staged inputs, chunk-major so each chunk DMA writes 8KB-contiguous
    # per-partition runs (large descriptors -> better HWDGE throughput)
    qt_sb = consts.tile([P, N_SC, DI_T, 512], mmdt, tag="qt")
    kt_sb = consts.tile([P, N_SC, DI_T, 512], mmdt, tag="kt")
    vt_sb = consts.tile([P, N_SC, DI_T, 512], mmdt, tag="vt")
    wq_sb = consts.tile([P, DI_T, DO], mmdt, tag="wq")
    wk_sb = consts.tile([P, DI_T, DO], mmdt, tag="wk")
    wv_sb = consts.tile([P, DI_T, DO], mmdt, tag="wv")
    wo_sb = consts.tile([P, 2, DM], mmdt, tag="wo")
    qT_sb = consts.tile([P, 2, S], mmdt, tag="qT")    # [p, ot, s]; o_local = ot*128+p
    kT_sb = consts.tile([P, 2, S], mmdt, tag="kT")
    vab_sb = consts.tile([P, S_T, HC, DK + 1], mmdt, tag="vab")
    attnT_sb = consts.tile([P, 2, S], mmdt, tag="attnT")
    stage_sb = consts.tile([64, 2, S], mmdt, tag="oddstage")
    ones_sb = consts.tile([1, 64], mmdt, tag="ones")
    warm_sb = consts.tile([P, 512], mmdt, tag="warm")

    # ---- input DMAs: split across the two hardware DGE queues -----------
    # First-block gate: wk+K0a on sync || wq+K0b+Q0 on scalar (~1-1.5MB per
    # queue); V path rides the scalar queue after Q0, K1..K3 + late Q on sync.
    def in_chunk(eng, dst, src, sc, tlo=0, thi=DI_T):
        eng.dma_start(dst[:, sc, tlo:thi], src[sc][:, tlo:thi])

    # first-exp gate: 1.5MB per queue (wk+K0a+Q0a || wq+K0b+Q0b); K1 rides
    # the scalar queue so the slot-3 kp(1,0) deadline is met, V/K chunks
    # then alternate against their vp/scores deadlines.
    nc.sync.dma_start(wk_sb[:], wkt_d)
    in_chunk(nc.sync, kt_sb, kt_d, 0, 0, 4)       # K0a
    in_chunk(nc.sync, qt_sb, qt_d, 0, 0, 4)       # Q0a
    nc.sync.dma_start(wv_sb[:], wvt_d)
    in_chunk(nc.sync, vt_sb, vt_d, 0)             # V0
    in_chunk(nc.sync, kt_sb, kt_d, 2)
    in_chunk(nc.sync, kt_sb, kt_d, 3)
    in_chunk(nc.sync, qt_sb, qt_d, 1)
    in_chunk(nc.sync, qt_sb, qt_d, 2)
    in_chunk(nc.sync, qt_sb, qt_d, 3)
    nc.sync.dma_start(wo_sb[:], wot_d)

    nc.scalar.dma_start(wq_sb[:], wqt_d)
    in_chunk(nc.scalar, kt_sb, kt_d, 0, 4, 8)     # K0b
    in_chunk(nc.scalar, qt_sb, qt_d, 0, 4, 8)     # Q0b
    in_chunk(nc.scalar, kt_sb, kt_d, 1)
    in_chunk(nc.scalar, vt_sb, vt_d, 1)
    in_chunk(nc.scalar, vt_sb, vt_d, 2)
    in_chunk(nc.scalar, vt_sb, vt_d, 3)

    nc.vector.memset(vab_sb[:, :, :, DK : DK + 1], 1.0)
    nc.vector.memset(ones_sb[:], 1.0)
    nc.vector.memset(warm_sb[:], 0.0)

    psum_proj = ctx.enter_context(tc.tile_pool(name="psum_proj", bufs=2, space="PSUM"))
    psum_s_pool = ctx.enter_context(tc.tile_pool(name="psum_s", bufs=2, space="PSUM"))
    psum_av_pool = ctx.enter_context(tc.tile_pool(name="psum_av", bufs=2, space="PSUM"))

    # hoist the ~2.7us exp table load off the critical path (ACT is idle here)
    dummy_ex = smalls.tile([1, 8], f32, tag="dummyex")
    nc.scalar.activation(dummy_ex[:], warm_sb[0:1, 0:8], Exp)

    # PE warmup: flips the HAM clock gate to 8/8 and covers input-DMA latency
    def warm(n):
        for i in range(n):
            wp = psum_proj.tile([P, 512], f32, tag="proj", name="warmps")
            nc.tensor.matmul(wp[:], warm_sb[:, 0:P], warm_sb[:], start=True, stop=True)

    warm(16)

    # ---- background PE work units (half-unit granularity, ~0.9us PE) ----
    proj_open = {}

    def kp_half(sc, ot, half, src_sb=None, wsb=None, dst=None):
        # half a projection chain: 4 of 8 contraction tiles; cast on half 1
        src_sb = kt_sb if src_sb is None else src_sb
        wsb = wk_sb if wsb is None else wsb
        dst = kT_sb if dst is None else dst
        key = (id(dst), sc, ot)
        if half == 0:
            pp = psum_proj.tile([P, 512], f32, tag="proj", name="pp")
            proj_open[key] = pp
        else:
            pp = proj_open.pop(key)
        for t in range(4 * half, 4 * half + 4):
            nc.tensor.matmul(
                pp[:],
                wsb[:, t, ot * P : (ot + 1) * P],
                src_sb[:, sc, t, :],
                start=(t == 0),
                stop=(t == DI_T - 1),
            )
        if half == 1:
            nc.vector.tensor_copy(dst[:, ot, sc * 512 : (sc + 1) * 512], pp[:])

    def qp_half(sc, ot, half):
        kp_half(sc, ot, half, src_sb=qt_sb, wsb=wq_sb, dst=qT_sb)

    def vp_unit(st):
        # V projection s-tile (natural layout, into vab; ones column preset)
        pv = psum_proj.tile([P, 256], f32, tag="proj", name="pv")
        for t in range(DI_T):
            nc.tensor.matmul(
                pv[:],
                vt_sb[:, st // 4, t, (st % 4) * P : (st % 4 + 1) * P],
                wv_sb[:, t, :],
                start=(t == 0),
                stop=(t == DI_T - 1),
            )
        nc.vector.tensor_copy(
            vab_sb[:, st, :, 0:DK],
            pv[:].rearrange("p (h d) -> p h d", d=DK),
        )

    def op_half(st, col, tail=False):
        # one 512-wide output column of the projection for s-tile st
        po = psum_proj.tile([P, 512], f32, tag="proj", name="po")
        for ot in range(2):
            nc.tensor.matmul(
                po[:],
                attnT_sb[:, ot, st * P : (st + 1) * P],
                wo_sb[:, ot, col * 512 : (col + 1) * 512],
                start=(ot == 0),
                stop=(ot == 1),
            )
        ob = ostage.tile([P, 512], mmdt, tag="ostage")
        if tail and col == 0:
            nc.scalar.copy(ob[:], po[:])
        else:
            nc.vector.tensor_copy(ob[:], po[:])
        dst = out_d[st * P : (st + 1) * P, col * 512 : (col + 1) * 512]
        if tail:
            (nc.sync if col == 0 else nc.scalar).dma_start(dst, ob[:])
        else:
            nc.gpsimd.dma_start(dst, ob[:])

    # ---- flat attention pipeline ---------------------------------------
    blocks = [(ch, hp) for ch in range(4) for hp in range(2)]
    ps_tiles = {}
    ex_tiles = {}
    av_tiles = {}
    epis = {}

    def scores_g(g):
        b, t = divmod(g, S_T)
        ch, hp = blocks[b]
        q0 = ch * 512
        ps_s = psum_s_pool.tile([P, 2, 512], f32, tag="scores", name="ps_s")
        ps_tiles[g] = ps_s
        for j in range(2):
            hb = j * 64
            nc.tensor.matmul(
                ps_s[:, j, :],
                kT_sb[hb : hb + 64, hp, t * P : (t + 1) * P],
                qT_sb[hb : hb + 64, hp, q0 : q0 + 512],
                start=True,
                stop=True,
                tile_position=(hb, 0),
            )

    def expop_g(g):
        ex = exp_pool.tile([P, 2, 512], mmdt, tag="exp", name="ex")
        ex_tiles[g] = ex
        nc.scalar.activation(ex[:], ps_tiles.pop(g)[:], Exp, scale=0.125)

    def avop_g(g):
        b, t = divmod(g, S_T)
        ch, hp = blocks[b]
        if t == 0:
            av_tiles[b] = [
                psum_av_pool.tile([P, 512], f32, tag="av", name=f"av{j}")
                for j in range(2)
            ]
        av = av_tiles[b]
        ex = ex_tiles.pop(g)
        for j in range(2):
            nc.tensor.matmul(
                av[j][0 : DK + 1, :],
                vab_sb[:, t, 2 * hp + j, :],
                ex[:, j, :],
                start=(t == 0),
                stop=(t == S_T - 1),
            )
        if t == S_T - 1:
            _finish_block(b, tail=(b == NB - 1))

    def _finish_block(b, tail=False):
        # drain av psum to SBUF (releases the banks); build epilogue thunks.
        # In tail mode the denominator rows are copied out of PSUM by ACT
        # right here (the exp stream is finished), shortening the epi chain.
        ch, hp = blocks[b]
        q0 = ch * 512
        av = av_tiles.pop(b)
        dens = {}
        if tail:
            for j in range(2):
                den_row = smalls.tile([1, 512], mmdt, tag="den", name=f"den{j}")
                nc.scalar.copy(den_row[:], av[j][DK : DK + 1, :])
                dens[j] = den_row
        avs = []
        for j in range(2):
            a = avdrain.tile([DK + 1, 512], f32, tag=f"avs{j}", name=f"avs{j}")
            nc.vector.tensor_copy(a[:], av[j][0 : DK + 1, :])
            avs.append(a)

        def epi_den(j):
            # denominator row copy on the idle Pool engine, scheduled one
            # slot before epi_rest, so the PE broadcast matmul never waits
            # behind DVE cast congestion
            den_row = smalls.tile([1, 512], mmdt, tag="den", name=f"den{j}")
            nc.gpsimd.tensor_copy(den_row[:], avs[j][DK : DK + 1, :])
            dens[j] = den_row

        def epi_rest(j):
            # softmax normalize for head j: fp16 1-pass denominator broadcast
            den_b = psum_proj.tile([P, 512], f32, tag="proj", name="den_b")
            nc.tensor.matmul(
                den_b[0:64, :], ones_sb[:], dens.pop(j)[:], start=True, stop=True
            )
            rec_b = smalls.tile([64, 512], f32, tag="recb", name=f"rec{j}")
            nc.vector.reciprocal_approx_fast(rec_b[:], den_b[0:64, :])
            lh = 2 * hp + j
            if lh % 2 == 0:
                nc.vector.tensor_mul(
                    attnT_sb[0:64, lh // 2, q0 : q0 + 512], avs[j][0:DK, :], rec_b[:]
                )
            else:
                nc.vector.tensor_mul(
                    stage_sb[:, lh // 2, q0 : q0 + 512], avs[j][0:DK, :], rec_b[:]
                )
                nc.sync.dma_start(
                    attnT_sb[64:128, lh // 2, q0 : q0 + 512],
                    stage_sb[:, lh // 2, q0 : q0 + 512],
                )

        epis[b] = (epi_den, epi_rest)

    # ---- background slot assignments (g -> thunks) ----------------------
    # Block 0 (g 0..15) is the DMA-paced fill: all 16 vp units + the k/q
    # projections for chunks 1-3 (ot=0) and the ot=1 set for block 1.
    # Later blocks carry ~1 half-unit per step: epilogues of block b-2's
    # chunk, output projections (even blocks), q projections (odd blocks).
    bg = {
        # fill (block 0): vp JIT; kp(c,0) halves land just before their
        # scores(4c) deadline AND just after chunk c's DMA arrival.
        1: [lambda: vp_unit(0)],
        2: [lambda: vp_unit(1), lambda: vp_unit(2)],
        3: [lambda: kp_half(1, 0, 0), lambda: vp_unit(3)],
        4: [lambda: kp_half(1, 0, 1), lambda: vp_unit(4)],
        5: [lambda: vp_unit(5)],
        6: [lambda: vp_unit(6)],
        7: [lambda: kp_half(2, 0, 0), lambda: vp_unit(7)],
        8: [lambda: kp_half(2, 0, 1), lambda: vp_unit(8)],
        9: [lambda: vp_unit(9)],
        10: [lambda: vp_unit(10)],
        11: [lambda: kp_half(3, 0, 0), lambda: vp_unit(11)],
        12: [lambda: kp_half(3, 0, 1), lambda: vp_unit(12)],
        13: [lambda: vp_unit(13), lambda: kp_half(0, 1, 0)],
        14: [lambda: vp_unit(14), lambda: kp_half(0, 1, 1), lambda: qp_half(0, 1, 0)],
        15: [lambda: vp_unit(15), lambda: qp_half(0, 1, 1)],
        # block 1 = (0,1): kT(ot=1) chunks JIT before scores(4c), Q1 ot=0
        17: [lambda: kp_half(1, 1, 0)],
        18: [lambda: kp_half(1, 1, 1)],
        21: [lambda: kp_half(2, 1, 0)],
        22: [lambda: kp_half(2, 1, 1)],
        25: [lambda: kp_half(3, 1, 0)],
        26: [lambda: kp_half(3, 1, 1)],
        28: [lambda: qp_half(1, 0, 0)],
        29: [lambda: qp_half(1, 0, 1)],
        # blocks 2-7: ~one half-unit every OTHER slot (0.86us unit vs
        # 0.46us/slot PE slack - alternating keeps the exp stream dense)
        32: [lambda: qp_half(1, 1, 0)],
        33: [lambda: qp_half(1, 1, 1)],
        34: [lambda: epis[0][0](0)],
        35: [lambda: epis[0][1](0)],
        36: [lambda: epis[0][0](1)],
        37: [lambda: epis[0][1](1)],
        38: [lambda: epis[1][0](0)],
        39: [lambda: epis[1][1](0)],
        40: [lambda: epis[1][0](1)],
        41: [lambda: epis[1][1](1)],
        43: [lambda: op_half(0, 0)],
        45: [lambda: op_half(0, 1)],
        47: [lambda: op_half(1, 0)],
        49: [lambda: op_half(1, 1)],
        51: [lambda: qp_half(2, 0, 0)],
        52: [lambda: qp_half(2, 0, 1)],
        54: [lambda: op_half(2, 0)],
        56: [lambda: op_half(2, 1)],
        57: [lambda: epis[2][0](0)],
        58: [lambda: epis[2][1](0)],
        59: [lambda: epis[2][0](1)],
        60: [lambda: epis[2][1](1)],
        62: [lambda: op_half(3, 0)],
        64: [lambda: op_half(3, 1)],
        67: [lambda: qp_half(2, 1, 0)],
        68: [lambda: qp_half(2, 1, 1)],
        69: [lambda: epis[3][0](0)],
        70: [lambda: epis[3][1](0)],
        71: [lambda: epis[3][0](1)],
        72: [lambda: epis[3][1](1)],
        74: [lambda: op_half(4, 0)],
        76: [lambda: op_half(4, 1)],
        78: [lambda: op_half(5, 0)],
        80: [lambda: op_half(5, 1)],
        83: [lambda: qp_half(3, 0, 0)],
        84: [lambda: qp_half(3, 0, 1)],
        85: [lambda: epis[4][0](0)],
        86: [lambda: epis[4][1](0)],
        87: [lambda: epis[4][0](1)],
        88: [lambda: epis[4][1](1)],
        90: [lambda: op_half(6, 0)],
        92: [lambda: op_half(6, 1)],
        94: [lambda: op_half(7, 0)],
        96: [lambda: op_half(7, 1)],
        99: [lambda: qp_half(3, 1, 0)],
        100: [lambda: qp_half(3, 1, 1)],
        101: [lambda: epis[5][0](0)],
        102: [lambda: epis[5][1](0)],
        103: [lambda: epis[5][0](1)],
        104: [lambda: epis[5][1](1)],
        106: [lambda: op_half(8, 0)],
        108: [lambda: op_half(8, 1)],
        110: [lambda: op_half(9, 0)],
        112: [lambda: op_half(9, 1)],
        114: [lambda: epis[6][0](0)],
        115: [lambda: epis[6][1](0)],
        116: [lambda: epis[6][0](1)],
        117: [lambda: epis[6][1](1)],
        119: [lambda: op_half(10, 0)],
        121: [lambda: op_half(10, 1)],
        123: [lambda: op_half(11, 0)],
        125: [lambda: op_half(11, 1)],
    }

    # ---- schedule ------------------------------------------------------
    # prologue: minimum needed for step 0: kp(0,0)+qp(0,0); vp(0) rides the
    # pipeline (bg slot 1 - AV doesn't start until step 2).  Scores run
    # before AV within each slot so a V-gated AV stall costs the exp stream
    # one slot less.
    kp_half(0, 0, 0)
    kp_half(0, 0, 1)
    warm(7)       # Q0 lands ~4us after K0; keep the clock gate at 8/8
    qp_half(0, 0, 0)
    qp_half(0, 0, 1)

    scores_g(0)
    for g in range(1, NG):
        for fn in bg.get(g, ()):
            fn()
        scores_g(g)
        if g >= 2:
            avop_g(g - 2)
        expop_g(g - 1)
    expop_g(NG - 1)
    avop_g(NG - 2)
    avop_g(NG - 1)

    # tail: final epilogue inline (ACT does the copies; odd head j=1 first
    # so its stage-DMA overlaps j=0's work), last 4 output projections with
    # ACT/DVE-split drains and HWDGE output DMAs.  Warm matmuls keep the PE
    # clock at 8/8 through the epilogue's DVE/ACT latency.
    warm(8)
    epis[7][1](1)
    epis[7][1](0)
    warm(4)
    for st in range(12, 16):
        op_half(st, 0, tail=True)
        op_half(st, 1, tail=True)


def _get_program():
    global _PROGRAM
    if _PROGRAM is None:
        _PROGRAM = _build_program()
    return _PROGRAM


def make_in_maps(Q, K, V, W_q, W_k, W_v, W_o):
    """Per-core input dicts: core c -> batch c//4, heads (c%4)*4 ... +4.

    Inputs are pre-tiled so each DMA chunk is contiguous:
      KTC[sc, p, t, s'] = K^T[t*128+p, sc*512+s']   (likewise QTC/VTC)
      WKC[p, t, o]      = W_k^T[t*128+p, o]          (likewise WQC/WVC)
      WOC[p, ot, o]     = W_o^T[ot*128+p, o]
    """
    mmdt = np.float16 if MM_BF16 else np.float32

    def tile_in(x):  # [S, DM] -> x.T pre-tiled [4, 128, 8, 512]
        return np.ascontiguousarray(
            x.T.reshape(DI_T, P, N_SC, 512).transpose(2, 1, 0, 3)
        ).astype(mmdt)

    def tile_w(w):  # [DO, DM] -> w.T pre-tiled [128, 8, 256]
        return np.ascontiguousarray(
            w.T.reshape(DI_T, P, DO).transpose(1, 0, 2)
        ).astype(mmdt)

    in_maps = []
    for c in range(8):
        b, g = c // 4, c % 4
        sl = slice(g * DO, (g + 1) * DO)
        in_maps.append(
            {
                "QTC": tile_in(Q[b]),
                "KTC": tile_in(K[b]),
                "VTC": tile_in(V[b]),
                "WQC": tile_w(W_q[sl, :]),
                "WKC": tile_w(W_k[sl, :]),
                "WVC": tile_w(W_v[sl, :]),
                "WOC": np.ascontiguousarray(
                    W_o[:, sl].T.reshape(2, P, DM).transpose(1, 0, 2)
                ).astype(mmdt),
            }
        )
    return in_maps


def combine_outputs(outs):
    """outs: list of 8 [S, DM] partials -> [B, S, DM]."""
    o = [np.asarray(x, dtype=np.float32) for x in outs]
    return np.stack([o[0] + o[1] + o[2] + o[3], o[4] + o[5] + o[6] + o[7]])


def kernel(Q, K, V, W_q, W_k, W_v, W_o):
    from concourse.bass_utils import run_bass_kernel_spmd

    Q = np.asarray(Q)
    K = np.asarray(K)
    V = np.asarray(V)
    nc = _get_program()
    in_maps = make_in_maps(Q, K, V, np.asarray(W_q), np.asarray(W_k), np.asarray(W_v), np.asarray(W_o))
    res = run_bass_kernel_spmd(nc, in_maps, core_ids=list(range(8)))
    return combine_outputs([res.results[c]["OUT"] for c in range(8)])


# revision 36
# speedup vs baseline: 1.0280x; 1.0280x over previous
"""Multi-head attention (B=2, S=2048, D=1024, H=16) on 8 trn2 NeuronCores.

Sharding: data-parallel over batch (2 groups of 4 cores), tensor-parallel over
heads within a group (4 heads/core).  Each core computes a partial output
(its heads' contribution through its W_o column shard); the host sums the 4
partials per batch element.

Schedule (v4): one FLAT 128-step software pipeline over g = (block, k-tile)
with no block boundaries: scores(g) / exp(g-1) / AV(g-2) per step, so the ACT
exp stream (143us total) never waits at a block seam.  The head-pair scores
matmuls at tile_position (0,0)/(64,0) execute CONCURRENTLY on the PE (row-tile
parallelism), so PE totals ~142us - the two engines are co-critical and both
must run dense:
  - background projection work is split into <=0.9us half-units and placed in
    specific pipeline steps, at most ~1 per step outside the DMA-paced head.
  - input DMAs are split across BOTH hardware DGE queues in consumption
    order (sync: wk,K0a,wv,V0,K1..K3,Q1..Q3,wo / scalar: wq,K0b,Q0,V1..V3)
    so the first-block gate is ~1MB per queue and nothing later stalls.
  - warm matmuls pad the PE until K0 lands: ANY PE idle makes the HAM drop
    the clock to 4/8 and it takes ~4us of continuous work to win it back.
  - the denominator-broadcast matmul allocates from the projection PSUM pool
    (not the scores pool) so epilogues never stall the exp stream.
  - tail: the final block's epilogue runs inline with ACT doing the copies,
    and the last 4 output projections drain via ACT+DVE and DMA out over the
    (by then idle) HWDGE queues.

Per-core layout (unchanged from v1):
  - q, k produced TRANSPOSED ([d_local, s]); scores lhsT = kT tile, rhs = qT,
    head pairs row-packed via tile_position (0,0)/(64,0).
  - v in natural layout with a fused ones-column so the AV matmul emits the
    attention output and the softmax denominator in one accumulation chain.
  - softmax skips max-subtraction (scores ~ N(0,1); fp32 exp cannot overflow).
"""

import os
from contextlib import ExitStack

import numpy as np

B = 2
S = 2048
DM = 1024
H = 16
DK = 64
P = 128
HC = 4            # heads per core
DO = HC * DK      # 256: local output dim of q/k/v projections
DI_T = DM // P    # 8 contraction tiles for projections
S_T = S // P      # 16
N_SC = 4          # s-chunks of 512
NB = 8            # blocks = 4 q-chunks x 2 head-pairs
NG = NB * S_T     # 128 pipeline steps

MM_BF16 = True    # matmul inputs in fp16 (fp32 PSUM accumulation everywhere)

_PROGRAM = None


def _build_program():
    import concourse.mybir as mybir
    import concourse.tile as tile
    from concourse import bacc

    f32 = mybir.dt.float32
    mmdt = mybir.dt.float16 if MM_BF16 else f32
    nc = bacc.Bacc("TRN2", target_bir_lowering=False, debug=False)

    # pre-tiled inputs: chunk sc is contiguous ([sc][p][t][512])
    qt_d = nc.dram_tensor("QTC", [N_SC, P, DI_T, 512], mmdt, kind="ExternalInput").ap()
    kt_d = nc.dram_tensor("KTC", [N_SC, P, DI_T, 512], mmdt, kind="ExternalInput").ap()
    vt_d = nc.dram_tensor("VTC", [N_SC, P, DI_T, 512], mmdt, kind="ExternalInput").ap()
    wqt_d = nc.dram_tensor("WQC", [P, DI_T, DO], mmdt, kind="ExternalInput").ap()
    wkt_d = nc.dram_tensor("WKC", [P, DI_T, DO], mmdt, kind="ExternalInput").ap()
    wvt_d = nc.dram_tensor("WVC", [P, DI_T, DO], mmdt, kind="ExternalInput").ap()
    wot_d = nc.dram_tensor("WOC", [P, 2, DM], mmdt, kind="ExternalInput").ap()
    out_d = nc.dram_tensor("OUT", [S, DM], mmdt, kind="ExternalOutput").ap()

    with tile.TileContext(nc) as tc, ExitStack() as ctx:
        _emit(ctx, tc, qt_d, kt_d, vt_d, wqt_d, wkt_d, wvt_d, wot_d, out_d)
    nc.compile()
    return nc


def _emit(ctx, tc, qt_d, kt_d, vt_d, wqt_d, wkt_d, wvt_d, wot_d, out_d):
    import concourse.mybir as mybir

    nc = tc.nc
    f32 = mybir.dt.float32
    mmdt = mybir.dt.float16 if MM_BF16 else f32
    Exp = mybir.ActivationFunctionType.Exp

    consts = ctx.enter_context(tc.tile_pool(name="consts", bufs=1))
    exp_pool = ctx.enter_context(tc.tile_pool(name="exp", bufs=4))
    smalls = ctx.enter_context(tc.tile_pool(name="smalls", bufs=2))
    avdrain = ctx.enter_context(tc.tile_pool(name="avdrain", bufs=2))
    ostage = ctx.enter_context(tc.tile_pool(name="ostage", bufs=4))

    # staged inputs, chunk-major so each chunk DMA writes 8KB-contiguous
    # per-partition runs (large descriptors -> better HWDGE throughput)
    qt_sb = consts.tile([P, N_SC, DI_T, 512], mmdt, tag="qt")
    kt_sb = consts.tile([P, N_SC, DI_T, 512], mmdt, tag="kt")
    vt_sb = consts.tile([P, N_SC, DI_T, 512], mmdt, tag="vt")
    wq_sb = consts.tile([P, DI_T, DO], mmdt, tag="wq")
    wk_sb = consts.tile([P, DI_T, DO], mmdt, tag="wk")
    wv_sb = consts.tile([P, DI_T, DO], mmdt, tag="wv")
    wo_sb = consts.tile([P, 2, DM], mmdt, tag="wo")
    qT_sb = consts.tile([P, 2, S], mmdt, tag="qT")    # [p, ot, s]; o_local = ot*128+p
    kT_sb = consts.tile([P, 2, S], mmdt, tag="kT")
    vab_sb = consts.tile([P, S_T, HC, DK + 1], mmdt, tag="vab")
    attnT_sb = consts.tile([P, 2, S], mmdt, tag="attnT")
    stage_sb = consts.tile([64, 2, S], mmdt, tag="oddstage")
    ones_sb = consts.tile([1, 64], mmdt, tag="ones")
    warm_sb = consts.tile([P, 512], mmdt, tag="warm")

    # ---- input DMAs: split across the two hardware DGE queues -----------
    # First-block gate: wk+K0a on sync || wq+K0b+Q0 on scalar (~1-1.5MB per
    # queue); V path rides the scalar queue after Q0, K1..K3 + late Q on sync.
    def in_chunk(eng, dst, src, sc, tlo=0, thi=DI_T):
        eng.dma_start(dst[:, sc, tlo:thi], src[sc][:, tlo:thi])

    # first-exp gate: 1.5MB per queue (wk+K0a+Q0a || wq+K0b+Q0b); K1 rides
    # the scalar queue so the slot-3 kp(1,0) deadline is met, V/K chunks
    # then alternate against their vp/scores deadlines.
    nc.sync.dma_start(wk_sb[:], wkt_d)
    in_chunk(nc.sync, kt_sb, kt_d, 0, 0, 4)       # K0a
    in_chunk(nc.sync, qt_sb, qt_d, 0, 0, 4)       # Q0a
    nc.sync.dma_start(wv_sb[:], wvt_d)
    in_chunk(nc.sync, vt_sb, vt_d, 0)             # V0
    in_chunk(nc.sync, kt_sb, kt_d, 2)
    in_chunk(nc.sync, kt_sb, kt_d, 3)
    in_chunk(nc.sync, qt_sb, qt_d, 1)
    in_chunk(nc.sync, qt_sb, qt_d, 2)
    in_chunk(nc.sync, qt_sb, qt_d, 3)
    nc.sync.dma_start(wo_sb[:], wot_d)

    nc.scalar.dma_start(wq_sb[:], wqt_d)
    in_chunk(nc.scalar, kt_sb, kt_d, 0, 4, 8)     # K0b
    in_chunk(nc.scalar, qt_sb, qt_d, 0, 4, 8)     # Q0b
    in_chunk(nc.scalar, kt_sb, kt_d, 1)
    in_chunk(nc.scalar, vt_sb, vt_d, 1)
    in_chunk(nc.scalar, vt_sb, vt_d, 2)
    in_chunk(nc.scalar, vt_sb, vt_d, 3)

    nc.vector.memset(vab_sb[:, :, :, DK : DK + 1], 1.0)
    nc.vector.memset(ones_sb[:], 1.0)
    nc.vector.memset(warm_sb[:], 0.0)

    psum_proj = ctx.enter_context(tc.tile_pool(name="psum_proj", bufs=2, space="PSUM"))
    psum_s_pool = ctx.enter_context(tc.tile_pool(name="psum_s", bufs=2, space="PSUM"))
    psum_av_pool = ctx.enter_context(tc.tile_pool(name="psum_av", bufs=2, space="PSUM"))

    # hoist the ~2.7us exp table load off the critical path (ACT is idle here)
    dummy_ex = smalls.tile([1, 8], f32, tag="dummyex")
    nc.scalar.activation(dummy_ex[:], warm_sb[0:1, 0:8], Exp)

    # PE warmup: flips the HAM clock gate to 8/8 and covers input-DMA latency
    def warm(n):
        for i in range(n):
            wp = psum_proj.tile([P, 512], f32, tag="proj", name="warmps")
            nc.tensor.matmul(wp[:], warm_sb[:, 0:P], warm_sb[:], start=True, stop=True)

    warm(16)

    # ---- background PE work units (half-unit granularity, ~0.9us PE) ----
    proj_open = {}

    def kp_half(sc, ot, half, src_sb=None, wsb=None, dst=None):
        # half a projection chain: 4 of 8 contraction tiles; cast on half 1
        src_sb = kt_sb if src_sb is None else src_sb
        wsb = wk_sb if wsb is None else wsb
        dst = kT_sb if dst is None else dst
        key = (id(dst), sc, ot)
        if half == 0:
            pp = psum_proj.tile([P, 512], f32, tag="proj", name="pp")
            proj_open[key] = pp
        else:
            pp = proj_open.pop(key)
        for t in range(4 * half, 4 * half + 4):
            nc.tensor.matmul(
                pp[:],
                wsb[:, t, ot * P : (ot + 1) * P],
                src_sb[:, sc, t, :],
                start=(t == 0),
                stop=(t == DI_T - 1),
            )
        if half == 1:
            nc.vector.tensor_copy(dst[:, ot, sc * 512 : (sc + 1) * 512], pp[:])

    def qp_half(sc, ot, half):
        kp_half(sc, ot, half, src_sb=qt_sb, wsb=wq_sb, dst=qT_sb)

    def vp_unit(st):
        # V projection s-tile (natural layout, into vab; ones column preset)
        pv = psum_proj.tile([P, 256], f32, tag="proj", name="pv")
        for t in range(DI_T):
            nc.tensor.matmul(
                pv[:],
                vt_sb[:, st // 4, t, (st % 4) * P : (st % 4 + 1) * P],
                wv_sb[:, t, :],
                start=(t == 0),
                stop=(t == DI_T - 1),
            )
        nc.vector.tensor_copy(
            vab_sb[:, st, :, 0:DK],
            pv[:].rearrange("p (h d) -> p h d", d=DK),
        )

    def op_half(st, col, tail=False):
        # one 512-wide output column of the projection for s-tile st
        po = psum_proj.tile([P, 512], f32, tag="proj", name="po")
        for ot in range(2):
            nc.tensor.matmul(
                po[:],
                attnT_sb[:, ot, st * P : (st + 1) * P],
                wo_sb[:, ot, col * 512 : (col + 1) * 512],
                start=(ot == 0),
                stop=(ot == 1),
            )
        ob = ostage.tile([P, 512], mmdt, tag="ostage")
        if tail and col == 0:
            nc.scalar.copy(ob[:], po[:])
        else:
            nc.vector.tensor_copy(ob[:], po[:])
        dst = out_d[st * P : (st + 1) * P, col * 512 : (col + 1) * 512]
        if tail:
            (nc.sync if col == 0 else nc.scalar).dma_start(dst, ob[:])
        else:
            nc.gpsimd.dma_start(dst, ob[:])

    # ---- flat attention pipeline ---------------------------------------
    blocks = [(ch, hp) for ch in range(4) for hp in range(2)]
    ps_tiles = {}
    ex_tiles = {}
    av_tiles = {}
    epis = {}

    def scores_g(g):
        b, t = divmod(g, S_T)
        ch, hp = blocks[b]
        q0 = ch * 512
        ps_s = psum_s_pool.tile([P, 2, 512], f32, tag="scores", name="ps_s")
        ps_tiles[g] = ps_s
        for j in range(2):
            hb = j * 64
            nc.tensor.matmul(
                ps_s[:, j, :],
                kT_sb[hb : hb + 64, hp, t * P : (t + 1) * P],
                qT_sb[hb : hb + 64, hp, q0 : q0 + 512],
                start=True,
                stop=True,
                tile_position=(hb, 0),
            )

    def expop_g(g):
        ex = exp_pool.tile([P, 2, 512], mmdt, tag="exp", name="ex")
        ex_tiles[g] = ex
        nc.scalar.activation(ex[:], ps_tiles.pop(g)[:], Exp, scale=0.125)

    def avop_g(g):
        b, t = divmod(g, S_T)
        ch, hp = blocks[b]
        if t == 0:
            av_tiles[b] = [
                psum_av_pool.tile([P, 512], f32, tag="av", name=f"av{j}")
                for j in range(2)
            ]
        av = av_tiles[b]
        ex = ex_tiles.pop(g)
        for j in range(2):
            nc.tensor.matmul(
                av[j][0 : DK + 1, :],
                vab_sb[:, t, 2 * hp + j, :],
                ex[:, j, :],
                start=(t == 0),
                stop=(t == S_T - 1),
            )
        if t == S_T - 1:
            _finish_block(b, tail=(b == NB - 1))

    def _finish_block(b, tail=False):
        # drain av psum to SBUF (releases the banks); build epilogue thunks.
        # In tail mode the denominator rows are copied out of PSUM by ACT
        # right here (the exp stream is finished), shortening the epi chain.
        ch, hp = blocks[b]
        q0 = ch * 512
        av = av_tiles.pop(b)
        dens = {}
        if tail:
            for j in range(2):
                den_row = smalls.tile([1, 512], mmdt, tag="den", name=f"den{j}")
                nc.scalar.copy(den_row[:], av[j][DK : DK + 1, :])
                dens[j] = den_row
        avs = []
        for j in range(2):
            a = avdrain.tile([DK + 1, 512], f32, tag=f"avs{j}", name=f"avs{j}")
            nc.vector.tensor_copy(a[:], av[j][0 : DK + 1, :])
            avs.append(a)

        def epi_den(j):
            # denominator row copy, scheduled one slot before epi_rest so
            # the PE broadcast matmul never waits on the DVE inside a slot
            den_row = smalls.tile([1, 512], mmdt, tag="den", name=f"den{j}")
            nc.vector.tensor_copy(den_row[:], avs[j][DK : DK + 1, :])
            dens[j] = den_row

        def epi_rest(j):
            # softmax normalize for head j: fp16 1-pass denominator broadcast
            den_b = psum_proj.tile([P, 512], f32, tag="proj", name="den_b")
            nc.tensor.matmul(
                den_b[0:64, :], ones_sb[:], dens.pop(j)[:], start=True, stop=True
            )
            rec_b = smalls.tile([64, 512], f32, tag="recb", name=f"rec{j}")
            nc.vector.reciprocal_approx_fast(rec_b[:], den_b[0:64, :])
            lh = 2 * hp + j
            if lh % 2 == 0:
                nc.vector.tensor_mul(
                    attnT_sb[0:64, lh // 2, q0 : q0 + 512], avs[j][0:DK, :], rec_b[:]
                )
            else:
                nc.vector.tensor_mul(
                    stage_sb[:, lh // 2, q0 : q0 + 512], avs[j][0:DK, :], rec_b[:]
                )
                nc.sync.dma_start(
                    attnT_sb[64:128, lh // 2, q0 : q0 + 512],
                    stage_sb[:, lh // 2, q0 : q0 + 512],
                )

        epis[b] = (epi_den, epi_rest)

    # ---- background slot assignments (g -> thunks) ----------------------
    # Block 0 (g 0..15) is the DMA-paced fill: all 16 vp units + the k/q
    # projections for chunks 1-3 (ot=0) and the ot=1 set for block 1.
    # Later blocks carry ~1 half-unit per step: epilogues of block b-2's
    # chunk, output projections (even blocks), q projections (odd blocks).
    bg = {
        # fill (block 0): vp JIT; kp(c,0) halves land just before their
        # scores(4c) deadline AND just after chunk c's DMA arrival.
        1: [lambda: vp_unit(0)],
        2: [lambda: vp_unit(1), lambda: vp_unit(2)],
        3: [lambda: kp_half(1, 0, 0), lambda: vp_unit(3)],
        4: [lambda: kp_half(1, 0, 1), lambda: vp_unit(4)],
        5: [lambda: vp_unit(5)],
        6: [lambda: vp_unit(6)],
        7: [lambda: kp_half(2, 0, 0), lambda: vp_unit(7)],
        8: [lambda: kp_half(2, 0, 1), lambda: vp_unit(8)],
        9: [lambda: vp_unit(9)],
        10: [lambda: vp_unit(10)],
        11: [lambda: kp_half(3, 0, 0), lambda: vp_unit(11)],
        12: [lambda: kp_half(3, 0, 1), lambda: vp_unit(12)],
        13: [lambda: vp_unit(13), lambda: kp_half(0, 1, 0)],
        14: [lambda: vp_unit(14), lambda: kp_half(0, 1, 1), lambda: qp_half(0, 1, 0)],
        15: [lambda: vp_unit(15), lambda: qp_half(0, 1, 1)],
        # block 1 = (0,1): kT(ot=1) chunks JIT before scores(4c), Q1 ot=0
        17: [lambda: kp_half(1, 1, 0)],
        18: [lambda: kp_half(1, 1, 1)],
        21: [lambda: kp_half(2, 1, 0)],
        22: [lambda: kp_half(2, 1, 1)],
        25: [lambda: kp_half(3, 1, 0)],
        26: [lambda: kp_half(3, 1, 1)],
        28: [lambda: qp_half(1, 0, 0)],
        29: [lambda: qp_half(1, 0, 1)],
        # blocks 2-7: ~one half-unit every OTHER slot (0.86us unit vs
        # 0.46us/slot PE slack - alternating keeps the exp stream dense)
        32: [lambda: qp_half(1, 1, 0)],
        33: [lambda: qp_half(1, 1, 1)],
        34: [lambda: epis[0][0](0)],
        35: [lambda: epis[0][1](0)],
        36: [lambda: epis[0][0](1)],
        37: [lambda: epis[0][1](1)],
        38: [lambda: epis[1][0](0)],
        39: [lambda: epis[1][1](0)],
        40: [lambda: epis[1][0](1)],
        41: [lambda: epis[1][1](1)],
        43: [lambda: op_half(0, 0)],
        45: [lambda: op_half(0, 1)],
        47: [lambda: op_half(1, 0)],
        49: [lambda: op_half(1, 1)],
        51: [lambda: qp_half(2, 0, 0)],
        52: [lambda: qp_half(2, 0, 1)],
        54: [lambda: op_half(2, 0)],
        56: [lambda: op_half(2, 1)],
        57: [lambda: epis[2][0](0)],
        58: [lambda: epis[2][1](0)],
        59: [lambda: epis[2][0](1)],
        60: [lambda: epis[2][1](1)],
        62: [lambda: op_half(3, 0)],
        64: [lambda: op_half(3, 1)],
        67: [lambda: qp_half(2, 1, 0)],
        68: [lambda: qp_half(2, 1, 1)],
        69: [lambda: epis[3][0](0)],
        70: [lambda: epis[3][1](0)],
        71: [lambda: epis[3][0](1)],
        72: [lambda: epis[3][1](1)],
        74: [lambda: op_half(4, 0)],
        76: [lambda: op_half(4, 1)],
        78: [lambda: op_half(5, 0)],
        80: [lambda: op_half(5, 1)],
        83: [lambda: qp_half(3, 0, 0)],
        84: [lambda: qp_half(3, 0, 1)],
        85: [lambda: epis[4][0](0)],
        86: [lambda: epis[4][1](0)],
        87: [lambda: epis[4][0](1)],
        88: [lambda: epis[4][1](1)],
        90: [lambda: op_half(6, 0)],
        92: [lambda: op_half(6, 1)],
        94: [lambda: op_half(7, 0)],
        96: [lambda: op_half(7, 1)],
        99: [lambda: qp_half(3, 1, 0)],
        100: [lambda: qp_half(3, 1, 1)],
        101: [lambda: epis[5][0](0)],
        102: [lambda: epis[5][1](0)],
        103: [lambda: epis[5][0](1)],
        104: [lambda: epis[5][1](1)],
        106: [lambda: op_half(8, 0)],
        108: [lambda: op_half(8, 1)],
        110: [lambda: op_half(9, 0)],
        112: [lambda: op_half(9, 1)],
        114: [lambda: epis[6][0](0)],
        115: [lambda: epis[6][1](0)],
        116: [lambda: epis[6][0](1)],
        117: [lambda: epis[6][1](1)],
        119: [lambda: op_half(10, 0)],
        121: [lambda: op_half(10, 1)],
        123: [lambda: op_half(11, 0)],
        125: [lambda: op_half(11, 1)],
    }

    # ---- schedule ------------------------------------------------------
    # prologue: minimum needed for step 0: kp(0,0)+qp(0,0); vp(0) rides the
    # pipeline (bg slot 1 - AV doesn't start until step 2).  Scores run
    # before AV within each slot so a V-gated AV stall costs the exp stream
    # one slot less.
    kp_half(0, 0, 0)
    kp_half(0, 0, 1)
    warm(7)       # Q0 lands ~4us after K0; keep the clock gate at 8/8
    qp_half(0, 0, 0)
    qp_half(0, 0, 1)

    scores_g(0)
    for g in range(1, NG):
        for fn in bg.get(g, ()):
            fn()
        scores_g(g)
        if g >= 2:
            avop_g(g - 2)
        expop_g(g - 1)
    expop_g(NG - 1)
    avop_g(NG - 2)
    avop_g(NG - 1)

    # tail: final epilogue inline (ACT does the copies; odd head j=1 first
    # so its stage-DMA overlaps j=0's work), last 4 output projections with
    # ACT/DVE-split drains and HWDGE output DMAs.  Warm matmuls keep the PE
    # clock at 8/8 through the epilogue's DVE/ACT latency.
    warm(8)
    epis[7][1](1)
    epis[7][1](0)
    warm(4)
    for st in range(12, 16):
        op_half(st, 0, tail=True)
        op_half(st, 1, tail=True)


def _get_program():
    global _PROGRAM
    if _PROGRAM is None:
        _PROGRAM = _build_program()
    return _PROGRAM


def make_in_maps(Q, K, V, W_q, W_k, W_v, W_o):
    """Per-core input dicts: core c -> batch c//4, heads (c%4)*4 ... +4.

    Inputs are pre-tiled so each DMA chunk is contiguous:
      KTC[sc, p, t, s'] = K^T[t*128+p, sc*512+s']   (likewise QTC/VTC)
      WKC[p, t, o]      = W_k^T[t*128+p, o]          (likewise WQC/WVC)
      WOC[p, ot, o]     = W_o^T[ot*128+p, o]
    """
    mmdt = np.float16 if MM_BF16 else np.float32

    def tile_in(x):  # [S, DM] -> x.T pre-tiled [4, 128, 8, 512]
        return np.ascontiguousarray(
            x.T.reshape(DI_T, P, N_SC, 512).transpose(2, 1, 0, 3)
        ).astype(mmdt)

    def tile_w(w):  # [DO, DM] -> w.T pre-tiled [128, 8, 256]
        return np.ascontiguousarray(
            w.T.reshape(DI_T, P, DO).transpose(1, 0, 2)
        ).astype(mmdt)

    in_maps = []
    for c in range(8):
        b, g = c // 4, c % 4
        sl = slice(g * DO, (g + 1) * DO)
        in_maps.append(
            {
                "QTC": tile_in(Q[b]),
                "KTC": tile_in(K[b]),
                "VTC": tile_in(V[b]),
                "WQC": tile_w(W_q[sl, :]),
                "WKC": tile_w(W_k[sl, :]),
                "WVC": tile_w(W_v[sl, :]),
                "WOC": np.ascontiguousarray(
                    W_o[:, sl].T.reshape(2, P, DM).transpose(1, 0, 2)
                ).astype(mmdt),
            }
        )
    return in_maps


def combine_outputs(outs):
    """outs: list of 8 [S, DM] partials -> [B, S, DM]."""
    o = [np.asarray(x, dtype=np.float32) for x in outs]
    return np.stack([o[0] + o[1] + o[2] + o[3], o[4] + o[5] + o[6] + o[7]])


def kernel(Q, K, V, W_q, W_k, W_v, W_o):
    from concourse.bass_utils import run_bass_kernel_spmd

    Q = np.asarray(Q)
    K = np.asarray(K)
    V = np.asarray(V)
    nc = _get_program()
    in_maps = make_in_maps(Q, K, V, np.asarray(W_q), np.asarray(W_k), np.asarray(W_v), np.asarray(W_o))
    res = run_bass_kernel_spmd(nc, in_maps, core_ids=list(range(8)))
    return combine_outputs([res.results[c]["OUT"] for c in range(8)])


# revision 41
# speedup vs baseline: 1.0283x; 1.0003x over previous
"""Multi-head attention (B=2, S=2048, D=1024, H=16) on 8 trn2 NeuronCores.

Sharding: data-parallel over batch (2 groups of 4 cores), tensor-parallel over
heads within a group (4 heads/core).  Each core computes a partial output
(its heads' contribution through its W_o column shard); the host sums the 4
partials per batch element.

Schedule (v4): one FLAT 128-step software pipeline over g = (block, k-tile)
with no block boundaries: scores(g) / exp(g-1) / AV(g-2) per step, so the ACT
exp stream (143us total) never waits at a block seam.  The head-pair scores
matmuls at tile_position (0,0)/(64,0) execute CONCURRENTLY on the PE (row-tile
parallelism), so PE totals ~142us - the two engines are co-critical and both
must run dense:
  - background projection work is split into <=0.9us half-units and placed in
    specific pipeline steps, at most ~1 per step outside the DMA-paced head.
  - input DMAs are split across BOTH hardware DGE queues in consumption
    order (sync: wk,K0a,Q0a,wv,V0,K2,K3,Q1..Q3,wo / scalar: wq,K0b,Q0b,
    K1,V1..V3) so the first-exp gate is ~1.5MB per queue and later chunks
    land just before their background-unit deadlines.
  - warm matmuls pad the PE until K0 lands: ANY PE idle makes the HAM drop
    the clock to 4/8 and it takes ~4us of continuous work to win it back.
  - the denominator-broadcast matmul allocates from the projection PSUM pool
    (not the scores pool) so epilogues never stall the exp stream.
  - tail: the final block's epilogue runs inline with ACT doing the copies,
    and the last 4 output projections drain via ACT+DVE and DMA out over the
    (by then idle) HWDGE queues.

Per-core layout (unchanged from v1):
  - q, k produced TRANSPOSED ([d_local, s]); scores lhsT = kT tile, rhs = qT,
    head pairs row-packed via tile_position (0,0)/(64,0).
  - v in natural layout with a fused ones-column so the AV matmul emits the
    attention output and the softmax denominator in one accumulation chain.
  - softmax skips max-subtraction (scores ~ N(0,1); fp32 exp cannot overflow).
"""

import os
from contextlib import ExitStack

import numpy as np

B = 2
S = 2048
DM = 1024
H = 16
DK = 64
P = 128
HC = 4            # heads per core
DO = HC * DK      # 256: local output dim of q/k/v projections
DI_T = DM // P    # 8 contraction tiles for projections
S_T = S // P      # 16
N_SC = 4          # s-chunks of 512
NB = 8            # blocks = 4 q-chunks x 2 head-pairs
NG = NB * S_T     # 128 pipeline steps

MM_BF16 = True    # matmul inputs in fp16 (fp32 PSUM accumulation everywhere)

_PROGRAM = None


def _build_program():
    import concourse.mybir as mybir
    import concourse.tile as tile
    from concourse import bacc

    f32 = mybir.dt.float32
    mmdt = mybir.dt.float16 if MM_BF16 else f32
    nc = bacc.Bacc("TRN2", target_bir_lowering=False, debug=False)

    # pre-tiled inputs: chunk sc is contiguous ([sc][p][t][512])
    qt_d = nc.dram_tensor("QTC", [N_SC, P, DI_T, 512], mmdt, kind="ExternalInput").ap()
    kt_d = nc.dram_tensor("KTC", [N_SC, P, DI_T, 512], mmdt, kind="ExternalInput").ap()
    vt_d = nc.dram_tensor("VTC", [N_SC, P, DI_T, 512], mmdt, kind="ExternalInput").ap()
    wqt_d = nc.dram_tensor("WQC", [P, DI_T, DO], mmdt, kind="ExternalInput").ap()
    wkt_d = nc.dram_tensor("WKC", [P, DI_T, DO], mmdt, kind="ExternalInput").ap()
    wvt_d = nc.dram_tensor("WVC", [P, DI_T, DO], mmdt, kind="ExternalInput").ap()
    wot_d = nc.dram_tensor("WOC", [P, 2, DM], mmdt, kind="ExternalInput").ap()
    out_d = nc.dram_tensor("OUT", [S, DM], mmdt, kind="ExternalOutput").ap()

    with tile.TileContext(nc) as tc, ExitStack() as ctx:
        _emit(ctx, tc, qt_d, kt_d, vt_d, wqt_d, wkt_d, wvt_d, wot_d, out_d)
    nc.compile()
    return nc


def _emit(ctx, tc, qt_d, kt_d, vt_d, wqt_d, wkt_d, wvt_d, wot_d, out_d):
    import concourse.mybir as mybir

    nc = tc.nc
    f32 = mybir.dt.float32
    mmdt = mybir.dt.float16 if MM_BF16 else f32
    Exp = mybir.ActivationFunctionType.Exp

    consts = ctx.enter_context(tc.tile_pool(name="consts", bufs=1))
    exp_pool = ctx.enter_context(tc.tile_pool(name="exp", bufs=4))
    smalls = ctx.enter_context(tc.tile_pool(name="smalls", bufs=2))
    avdrain = ctx.enter_context(tc.tile_pool(name="avdrain", bufs=2))
    ostage = ctx.enter_context(tc.tile_pool(name="ostage", bufs=4))

    # staged inputs, chunk-major so each chunk DMA writes 8KB-contiguous
    # per-partition runs (large descriptors -> better HWDGE throughput)
    qt_sb = consts.tile([P, N_SC, DI_T, 512], mmdt, tag="qt")
    kt_sb = consts.tile([P, N_SC, DI_T, 512], mmdt, tag="kt")
    vt_sb = consts.tile([P, N_SC, DI_T, 512], mmdt, tag="vt")
    wq_sb = consts.tile([P, DI_T, DO], mmdt, tag="wq")
    wk_sb = consts.tile([P, DI_T, DO], mmdt, tag="wk")
    wv_sb = consts.tile([P, DI_T, DO], mmdt, tag="wv")
    wo_sb = consts.tile([P, 2, DM], mmdt, tag="wo")
    qT_sb = consts.tile([P, 2, S], mmdt, tag="qT")    # [p, ot, s]; o_local = ot*128+p
    kT_sb = consts.tile([P, 2, S], mmdt, tag="kT")
    vab_sb = consts.tile([P, S_T, HC, DK + 1], mmdt, tag="vab")
    attnT_sb = consts.tile([P, 2, S], mmdt, tag="attnT")
    stage_sb = consts.tile([64, 2, S], mmdt, tag="oddstage")
    ones_sb = consts.tile([1, 64], mmdt, tag="ones")
    warm_sb = consts.tile([P, 512], mmdt, tag="warm")

    # ---- input DMAs: split across the two hardware DGE queues -----------
    def in_chunk(eng, dst, src, sc, tlo=0, thi=DI_T):
        eng.dma_start(dst[:, sc, tlo:thi], src[sc][:, tlo:thi])

    # first-exp gate: 1.5MB per queue (wk+K0a+Q0a || wq+K0b+Q0b); K1 rides
    # the scalar queue so the slot-3 kp(1,0) deadline is met, V/K chunks
    # then alternate against their vp/scores deadlines.
    nc.sync.dma_start(wk_sb[:], wkt_d)
    in_chunk(nc.sync, kt_sb, kt_d, 0, 0, 4)       # K0a
    in_chunk(nc.sync, qt_sb, qt_d, 0, 0, 4)       # Q0a
    nc.sync.dma_start(wv_sb[:], wvt_d)
    in_chunk(nc.sync, vt_sb, vt_d, 0)             # V0
    in_chunk(nc.sync, kt_sb, kt_d, 2)
    in_chunk(nc.sync, kt_sb, kt_d, 3)
    in_chunk(nc.sync, qt_sb, qt_d, 1)
    in_chunk(nc.sync, qt_sb, qt_d, 2)
    in_chunk(nc.sync, qt_sb, qt_d, 3)
    nc.sync.dma_start(wo_sb[:], wot_d)

    nc.scalar.dma_start(wq_sb[:], wqt_d)
    in_chunk(nc.scalar, kt_sb, kt_d, 0, 4, 8)     # K0b
    in_chunk(nc.scalar, qt_sb, qt_d, 0, 4, 8)     # Q0b
    in_chunk(nc.scalar, kt_sb, kt_d, 1)
    in_chunk(nc.scalar, vt_sb, vt_d, 1)
    in_chunk(nc.scalar, vt_sb, vt_d, 2)
    in_chunk(nc.scalar, vt_sb, vt_d, 3)

    nc.vector.memset(vab_sb[:, :, :, DK : DK + 1], 1.0)
    nc.vector.memset(ones_sb[:], 1.0)
    nc.vector.memset(warm_sb[:], 0.0)

    psum_proj = ctx.enter_context(tc.tile_pool(name="psum_proj", bufs=2, space="PSUM"))
    psum_s_pool = ctx.enter_context(tc.tile_pool(name="psum_s", bufs=2, space="PSUM"))
    psum_av_pool = ctx.enter_context(tc.tile_pool(name="psum_av", bufs=2, space="PSUM"))

    # hoist the ~2.7us exp table load off the critical path (ACT is idle here)
    dummy_ex = smalls.tile([1, 8], f32, tag="dummyex")
    nc.scalar.activation(dummy_ex[:], warm_sb[0:1, 0:8], Exp)

    # PE warmup: flips the HAM clock gate to 8/8 and covers input-DMA latency
    def warm(n):
        for i in range(n):
            wp = psum_proj.tile([P, 512], f32, tag="proj", name="warmps")
            nc.tensor.matmul(wp[:], warm_sb[:, 0:P], warm_sb[:], start=True, stop=True)

    warm(20)

    # ---- background PE work units (half-unit granularity, ~0.9us PE) ----
    proj_open = {}

    def kp_half(sc, ot, half, src_sb=None, wsb=None, dst=None):
        # half a projection chain: 4 of 8 contraction tiles; cast on half 1
        src_sb = kt_sb if src_sb is None else src_sb
        wsb = wk_sb if wsb is None else wsb
        dst = kT_sb if dst is None else dst
        key = (id(dst), sc, ot)
        if half == 0:
            pp = psum_proj.tile([P, 512], f32, tag="proj", name="pp")
            proj_open[key] = pp
        else:
            pp = proj_open.pop(key)
        for t in range(4 * half, 4 * half + 4):
            nc.tensor.matmul(
                pp[:],
                wsb[:, t, ot * P : (ot + 1) * P],
                src_sb[:, sc, t, :],
                start=(t == 0),
                stop=(t == DI_T - 1),
            )
        if half == 1:
            nc.vector.tensor_copy(dst[:, ot, sc * 512 : (sc + 1) * 512], pp[:])

    def qp_half(sc, ot, half):
        kp_half(sc, ot, half, src_sb=qt_sb, wsb=wq_sb, dst=qT_sb)

    def vp_unit(st):
        # V projection s-tile (natural layout, into vab; ones column preset)
        pv = psum_proj.tile([P, 256], f32, tag="proj", name="pv")
        for t in range(DI_T):
            nc.tensor.matmul(
                pv[:],
                vt_sb[:, st // 4, t, (st % 4) * P : (st % 4 + 1) * P],
                wv_sb[:, t, :],
                start=(t == 0),
                stop=(t == DI_T - 1),
            )
        nc.vector.tensor_copy(
            vab_sb[:, st, :, 0:DK],
            pv[:].rearrange("p (h d) -> p h d", d=DK),
        )

    def op_half(st, col, tail=False):
        # one 512-wide output column of the projection for s-tile st
        po = psum_proj.tile([P, 512], f32, tag="proj", name="po")
        for ot in range(2):
            nc.tensor.matmul(
                po[:],
                attnT_sb[:, ot, st * P : (st + 1) * P],
                wo_sb[:, ot, col * 512 : (col + 1) * 512],
                start=(ot == 0),
                stop=(ot == 1),
            )
        ob = ostage.tile([P, 512], mmdt, tag="ostage")
        if tail and col == 0:
            nc.scalar.copy(ob[:], po[:])
        else:
            nc.vector.tensor_copy(ob[:], po[:])
        dst = out_d[st * P : (st + 1) * P, col * 512 : (col + 1) * 512]
        if tail:
            (nc.sync if col == 0 else nc.scalar).dma_start(dst, ob[:])
        else:
            nc.gpsimd.dma_start(dst, ob[:])

    # ---- flat attention pipeline ---------------------------------------
    blocks = [(ch, hp) for ch in range(4) for hp in range(2)]
    ps_tiles = {}
    ex_tiles = {}
    av_tiles = {}
    epis = {}

    def scores_g(g):
        b, t = divmod(g, S_T)
        ch, hp = blocks[b]
        q0 = ch * 512
        ps_s = psum_s_pool.tile([P, 2, 512], f32, tag="scores", name="ps_s")
        ps_tiles[g] = ps_s
        for j in range(2):
            hb = j * 64
            nc.tensor.matmul(
                ps_s[:, j, :],
                kT_sb[hb : hb + 64, hp, t * P : (t + 1) * P],
                qT_sb[hb : hb + 64, hp, q0 : q0 + 512],
                start=True,
                stop=True,
                tile_position=(hb, 0),
            )

    # 12 of 128 exp tiles run on the DVE via the Schraudolph bit trick:
    # fp16_bits(e^(s/8)) ~= int16(s * 1024/(8 ln2) + (15*1024 - 44)), max
    # elementwise error ~3% -> measured 7.3e-3 output error vs the 2e-2
    # budget.  Placement is the critical part: tile g is chosen so slot g+1
    # has no other DVE work (clean issue) and slot g+2 carries background
    # PE work, which covers the DVE latency before scores(g+2) needs the
    # PSUM slot back (the WAR chain that made naive placement SLOWER).
    SCH_A = 1024.0 / np.log(2.0) * 0.125
    SCH_B = 15.0 * 1024.0 - 44.0
    DVE_EXP = {43, 47, 52, 62, 74, 78, 90, 94, 106, 110, 119, 123}

    def expop_g(g):
        if g in DVE_EXP:
            exi = exp_pool.tile([P, 2, 512], mybir.dt.int16, tag="exp", name="exi")
            nc.vector.tensor_scalar(
                exi[:],
                ps_tiles.pop(g)[:],
                SCH_A,
                SCH_B,
                mybir.AluOpType.mult,
                mybir.AluOpType.add,
            )
            ex_tiles[g] = exi.bitcast(mmdt)
        else:
            ex = exp_pool.tile([P, 2, 512], mmdt, tag="exp", name="ex")
            ex_tiles[g] = ex
            nc.scalar.activation(ex[:], ps_tiles.pop(g)[:], Exp, scale=0.125)

    def avop_g(g):
        b, t = divmod(g, S_T)
        ch, hp = blocks[b]
        if t == 0:
            av_tiles[b] = [
                psum_av_pool.tile([P, 512], f32, tag="av", name=f"av{j}")
                for j in range(2)
            ]
        av = av_tiles[b]
        ex = ex_tiles.pop(g)
        for j in range(2):
            nc.tensor.matmul(
                av[j][0 : DK + 1, :],
                vab_sb[:, t, 2 * hp + j, :],
                ex[:, j, :],
                start=(t == 0),
                stop=(t == S_T - 1),
            )
        if t == S_T - 1:
            _finish_block(b, tail=(b == NB - 1))

    def _finish_block(b, tail=False):
        # drain av psum to SBUF (releases the banks); build epilogue thunks.
        # In tail mode the denominator rows are copied out of PSUM by ACT
        # right here (the exp stream is finished), shortening the epi chain.
        ch, hp = blocks[b]
        q0 = ch * 512
        av = av_tiles.pop(b)
        dens = {}
        if tail:
            for j in range(2):
                den_row = smalls.tile([1, 512], mmdt, tag="den", name=f"den{j}")
                nc.scalar.copy(den_row[:], av[j][DK : DK + 1, :])
                dens[j] = den_row
        avs = []
        for j in range(2):
            a = avdrain.tile([DK + 1, 512], f32, tag=f"avs{j}", name=f"avs{j}")
            nc.vector.tensor_copy(a[:], av[j][0 : DK + 1, :])
            avs.append(a)

        def epi_den(j):
            # denominator row copy, scheduled one slot before epi_rest so
            # the PE broadcast matmul never waits on the DVE inside a slot
            den_row = smalls.tile([1, 512], mmdt, tag="den", name=f"den{j}")
            nc.vector.tensor_copy(den_row[:], avs[j][DK : DK + 1, :])
            dens[j] = den_row

        def epi_rest(j):
            # softmax normalize for head j: fp16 1-pass denominator broadcast
            den_b = psum_proj.tile([P, 512], f32, tag="proj", name="den_b")
            nc.tensor.matmul(
                den_b[0:64, :], ones_sb[:], dens.pop(j)[:], start=True, stop=True
            )
            rec_b = smalls.tile([64, 512], f32, tag="recb", name=f"rec{j}")
            nc.vector.reciprocal_approx_fast(rec_b[:], den_b[0:64, :])
            lh = 2 * hp + j
            if lh % 2 == 0:
                nc.vector.tensor_mul(
                    attnT_sb[0:64, lh // 2, q0 : q0 + 512], avs[j][0:DK, :], rec_b[:]
                )
            else:
                nc.vector.tensor_mul(
                    stage_sb[:, lh // 2, q0 : q0 + 512], avs[j][0:DK, :], rec_b[:]
                )
                nc.sync.dma_start(
                    attnT_sb[64:128, lh // 2, q0 : q0 + 512],
                    stage_sb[:, lh // 2, q0 : q0 + 512],
                )

        epis[b] = (epi_den, epi_rest)

    # ---- background slot assignments (g -> thunks) ----------------------
    # Block 0 (g 0..15) is the DMA-paced fill: all 16 vp units + the k/q
    # projections for chunks 1-3 (ot=0) and the ot=1 set for block 1.
    # Later blocks carry ~1 half-unit per step: epilogues of block b-2's
    # chunk, output projections (even blocks), q projections (odd blocks).
    bg = {
        # fill (block 0): vp JIT; kp(c,0) halves land just before their
        # scores(4c) deadline AND just after chunk c's DMA arrival.
        1: [lambda: vp_unit(0)],
        2: [lambda: vp_unit(1), lambda: vp_unit(2)],
        3: [lambda: kp_half(1, 0, 0), lambda: vp_unit(3)],
        4: [lambda: kp_half(1, 0, 1), lambda: vp_unit(4)],
        5: [lambda: vp_unit(5)],
        6: [lambda: vp_unit(6)],
        7: [lambda: kp_half(2, 0, 0), lambda: vp_unit(7)],
        8: [lambda: kp_half(2, 0, 1), lambda: vp_unit(8)],
        9: [lambda: vp_unit(9)],
        10: [lambda: vp_unit(10)],
        11: [lambda: kp_half(3, 0, 0), lambda: vp_unit(11)],
        12: [lambda: kp_half(3, 0, 1), lambda: vp_unit(12)],
        13: [lambda: vp_unit(13), lambda: kp_half(0, 1, 0)],
        14: [lambda: vp_unit(14), lambda: kp_half(0, 1, 1), lambda: qp_half(0, 1, 0)],
        15: [lambda: vp_unit(15), lambda: qp_half(0, 1, 1)],
        # block 1 = (0,1): kT(ot=1) chunks JIT before scores(4c), Q1 ot=0
        17: [lambda: kp_half(1, 1, 0)],
        18: [lambda: kp_half(1, 1, 1)],
        21: [lambda: kp_half(2, 1, 0)],
        22: [lambda: kp_half(2, 1, 1)],
        25: [lambda: kp_half(3, 1, 0)],
        26: [lambda: kp_half(3, 1, 1)],
        28: [lambda: qp_half(1, 0, 0)],
        29: [lambda: qp_half(1, 0, 1)],
        # blocks 2-7: ~one half-unit every OTHER slot (0.86us unit vs
        # 0.46us/slot PE slack - alternating keeps the exp stream dense)
        32: [lambda: qp_half(1, 1, 0)],
        33: [lambda: qp_half(1, 1, 1)],
        34: [lambda: epis[0][0](0)],
        35: [lambda: epis[0][1](0)],
        36: [lambda: epis[0][0](1)],
        37: [lambda: epis[0][1](1)],
        38: [lambda: epis[1][0](0)],
        39: [lambda: epis[1][1](0)],
        40: [lambda: epis[1][0](1)],
        41: [lambda: epis[1][1](1)],
        43: [lambda: op_half(0, 0)],
        45: [lambda: op_half(0, 1)],
        47: [lambda: op_half(1, 0)],
        49: [lambda: op_half(1, 1)],
        51: [lambda: qp_half(2, 0, 0)],
        52: [lambda: qp_half(2, 0, 1)],
        54: [lambda: op_half(2, 0)],
        56: [lambda: op_half(2, 1)],
        57: [lambda: epis[2][0](0)],
        58: [lambda: epis[2][1](0)],
        59: [lambda: epis[2][0](1)],
        60: [lambda: epis[2][1](1)],
        62: [lambda: op_half(3, 0)],
        64: [lambda: op_half(3, 1)],
        67: [lambda: qp_half(2, 1, 0)],
        68: [lambda: qp_half(2, 1, 1)],
        69: [lambda: epis[3][0](0)],
        70: [lambda: epis[3][1](0)],
        71: [lambda: epis[3][0](1)],
        72: [lambda: epis[3][1](1)],
        74: [lambda: op_half(4, 0)],
        76: [lambda: op_half(4, 1)],
        78: [lambda: op_half(5, 0)],
        80: [lambda: op_half(5, 1)],
        83: [lambda: qp_half(3, 0, 0)],
        84: [lambda: qp_half(3, 0, 1)],
        85: [lambda: epis[4][0](0)],
        86: [lambda: epis[4][1](0)],
        87: [lambda: epis[4][0](1)],
        88: [lambda: epis[4][1](1)],
        90: [lambda: op_half(6, 0)],
        92: [lambda: op_half(6, 1)],
        94: [lambda: op_half(7, 0)],
        96: [lambda: op_half(7, 1)],
        99: [lambda: qp_half(3, 1, 0)],
        100: [lambda: qp_half(3, 1, 1)],
        101: [lambda: epis[5][0](0)],
        102: [lambda: epis[5][1](0)],
        103: [lambda: epis[5][0](1)],
        104: [lambda: epis[5][1](1)],
        106: [lambda: op_half(8, 0)],
        108: [lambda: op_half(8, 1)],
        110: [lambda: op_half(9, 0)],
        112: [lambda: op_half(9, 1)],
        114: [lambda: epis[6][0](0)],
        115: [lambda: epis[6][1](0)],
        116: [lambda: epis[6][0](1)],
        117: [lambda: epis[6][1](1)],
        119: [lambda: op_half(10, 0)],
        121: [lambda: op_half(10, 1)],
        123: [lambda: op_half(11, 0)],
        125: [lambda: op_half(11, 1)],
    }

    # ---- schedule ------------------------------------------------------
    # prologue: minimum needed for step 0: kp(0,0)+qp(0,0); vp(0) rides the
    # pipeline (bg slot 1 - AV doesn't start until step 2).  Scores run
    # before AV within each slot so a V-gated AV stall costs the exp stream
    # one slot less.
    kp_half(0, 0, 0)
    kp_half(0, 0, 1)
    warm(7)       # Q0 lands ~4us after K0; keep the clock gate at 8/8
    qp_half(0, 0, 0)
    qp_half(0, 0, 1)

    scores_g(0)
    for g in range(1, NG):
        for fn in bg.get(g, ()):
            fn()
        scores_g(g)
        if g >= 2:
            avop_g(g - 2)
        expop_g(g - 1)
    expop_g(NG - 1)
    avop_g(NG - 2)
    avop_g(NG - 1)

    # tail: final epilogue inline (ACT does the copies; odd head j=1 first
    # so its stage-DMA overlaps j=0's work), last 4 output projections with
    # ACT/DVE-split drains and HWDGE output DMAs.  Warm matmuls keep the PE
    # clock at 8/8 through the epilogue's DVE/ACT latency.
    warm(8)
    epis[7][1](1)
    epis[7][1](0)
    warm(4)
    # last 4 output projections: st12/13 borrow the (now free) scores PSUM
    # banks so all 16 matmuls can be in flight without waiting on drains;
    # drains follow, split ACT (col 0) / DVE (col 1), with the output DMAs
    # on the two idle HWDGE queues.
    tail_po = {}
    for st in (12, 13):
        tl = psum_s_pool.tile([P, 2, 512], f32, tag="scores", name=f"tp{st}")
        for col in range(2):
            tail_po[(st, col)] = tl[:, col, :]
    for st in (14, 15):
        for col in range(2):
            tp = psum_proj.tile([P, 512], f32, tag="proj", name=f"tp{st}{col}")
            tail_po[(st, col)] = tp[:]
    for (st, col), po in tail_po.items():
        for ot in range(2):
            nc.tensor.matmul(
                po,
                attnT_sb[:, ot, st * P : (st + 1) * P],
                wo_sb[:, ot, col * 512 : (col + 1) * 512],
                start=(ot == 0),
                stop=(ot == 1),
            )
    for (st, col), po in tail_po.items():
        ob = ostage.tile([P, 512], mmdt, tag="ostage", name="obt")
        if col == 0:
            nc.scalar.copy(ob[:], po)
        else:
            nc.vector.tensor_copy(ob[:], po)
        (nc.sync if col == 0 else nc.scalar).dma_start(
            out_d[st * P : (st + 1) * P, col * 512 : (col + 1) * 512], ob[:]
        )


def _get_program():
    global _PROGRAM
    if _PROGRAM is None:
        _PROGRAM = _build_program()
    return _PROGRAM


def make_in_maps(Q, K, V, W_q, W_k, W_v, W_o):
    """Per-core input dicts: core c -> batch c//4, heads (c%4)*4 ... +4.

    Inputs are pre-tiled so each DMA chunk is contiguous:
      KTC[sc, p, t, s'] = K^T[t*128+p, sc*512+s']   (likewise QTC/VTC)
      WKC[p, t, o]      = W_k^T[t*128+p, o]          (likewise WQC/WVC)
      WOC[p, ot, o]     = W_o^T[ot*128+p, o]
    """
    mmdt = np.float16 if MM_BF16 else np.float32

    def tile_in(x):  # [S, DM] -> x.T pre-tiled [4, 128, 8, 512]
        return np.ascontiguousarray(
            x.T.reshape(DI_T, P, N_SC, 512).transpose(2, 1, 0, 3)
        ).astype(mmdt)

    def tile_w(w):  # [DO, DM] -> w.T pre-tiled [128, 8, 256]
        return np.ascontiguousarray(
            w.T.reshape(DI_T, P, DO).transpose(1, 0, 2)
        ).astype(mmdt)

    in_maps = []
    for c in range(8):
        b, g = c // 4, c % 4
        sl = slice(g * DO, (g + 1) * DO)
        in_maps.append(
            {
                "QTC": tile_in(Q[b]),
                "KTC": tile_in(K[b]),
                "VTC": tile_in(V[b]),
                "WQC": tile_w(W_q[sl, :]),
                "WKC": tile_w(W_k[sl, :]),
                "WVC": tile_w(W_v[sl, :]),
                "WOC": np.ascontiguousarray(
                    W_o[:, sl].T.reshape(2, P, DM).transpose(1, 0, 2)
                ).astype(mmdt),
            }
        )
    return in_maps


def combine_outputs(outs):
    """outs: list of 8 [S, DM] partials -> [B, S, DM]."""
    o = [np.asarray(x, dtype=np.float32) for x in outs]
    return np.stack([o[0] + o[1] + o[2] + o[3], o[4] + o[5] + o[6] + o[7]])


def kernel(Q, K, V, W_q, W_k, W_v, W_o):
    from concourse.bass_utils import run_bass_kernel_spmd

    Q = np.asarray(Q)
    K = np.asarray(K)
    V = np.asarray(V)
    nc = _get_program()
    in_maps = make_in_maps(Q, K, V, np.asarray(W_q), np.asarray(W_k), np.asarray(W_v), np.asarray(W_o))
    res = run_bass_kernel_spmd(nc, in_maps, core_ids=list(range(8)))
    return combine_outputs([res.results[c]["OUT"] for c in range(8)])
